# revision 7
# baseline (speedup 1.0000x reference)
"""Trainium2 Bass kernel for CrossAttention (B=4, T=2048, S=1024, C=1024, H=16).

Sharding: 8 cores = batch (4) x head-group (2 groups of 8 heads).
Each core computes the encoder MLP+LN for its batch (duplicated across the
2 head-group cores), its head-group's q/k/v, attention, and a partial
output projection. Host sums the two partial projections per batch + bp.

All matmuls run in bf16 (fp32 PSUM accumulation). Dataflow is transposed
(qT/kT/yT layouts) so no on-device transposes are needed except one bf16
DMA-transpose of the normalized encoder activations. Softmax denominators
come from a ones-column folded into the AV matmul; the attention mask and
1/sqrt(D) scale are folded into the exp bias / q-projection weights.
"""

import sys

sys.path.insert(0, "/opt/trn_rl_repo")

import numpy as np
import ml_dtypes
from contextlib import ExitStack

import concourse.bass as bass
import concourse.tile as tile
from concourse import bacc, mybir
from concourse.bass import ts

F32 = mybir.dt.float32
BF16 = mybir.dt.bfloat16
AF = mybir.ActivationFunctionType
ALU = mybir.AluOpType

# CoreSim doesn't implement the Gelu table; tests swap this to Identity to
# validate dataflow in sim. HW always uses the real erf-based Gelu.
GELU_FUNC = AF.Gelu

# Problem dims (fixed by the reference)
B, T, S, C, E, H, D = 4, 2048, 1024, 1024, 512, 16, 64
G = C // 2  # head-group width (8 heads * 64)
HL = G // D  # 8 local heads
P = 128
TCH = 512  # attention t-chunk
KC_C = C // P  # 8
KC_E = E // P  # 4
MC_G = G // P  # 4
SC = S // P  # 8
NTC = T // TCH  # 4
TB = T // P  # 16
NC2 = C // 512  # 2


def build_program():
    nc = bacc.Bacc("TRN2", target_bir_lowering=False, debug=False)

    def din(name, shape, dtype=BF16):
        return nc.dram_tensor(name, shape, dtype, kind="ExternalInput").ap()

    eeT = din("eeT", [KC_E, P, S])
    xT = din("xT", [KC_C, P, T])
    w1 = din("w1", [KC_E, P, C])
    w2 = din("w2", [KC_C, P, C])
    wq = din("wq", [KC_C, P, G])
    wk = din("wk", [KC_C, P, G])
    wv = din("wv", [KC_C, P, G])
    wp = din("wp", [MC_G, P, C])
    b1c = din("b1c", [P, KC_C], F32)
    b2r = din("b2r", [1, C], F32)
    bqc = din("bqc", [P, MC_G], F32)
    bkc = din("bkc", [P, MC_G], F32)
    bvr = din("bvr", [1, G], F32)
    mb = din("mb", [P, SC], F32)
    out = nc.dram_tensor("out", [TB, P, C], F32, kind="ExternalOutput").ap()

    def bcast(src_row, parts):
        # DRAM [1, N] row -> partition-broadcast AP [parts, N]
        return bass.AP(
            tensor=src_row.tensor,
            offset=src_row.offset,
            ap=[[0, parts]] + [list(x) for x in src_row.ap[1:]],
        )

    with tile.TileContext(nc) as tc, ExitStack() as ctx:
        const = ctx.enter_context(tc.tile_pool(name="const", bufs=1))
        persist = ctx.enter_context(tc.tile_pool(name="persist", bufs=1))
        psA = ctx.enter_context(tc.tile_pool(name="psA", bufs=3, space="PSUM"))
        psQK = ctx.enter_context(tc.tile_pool(name="psQK", bufs=3, space="PSUM"))
        psAV = ctx.enter_context(tc.tile_pool(name="psAV", bufs=2, space="PSUM"))
        dpool = ctx.enter_context(tc.tile_pool(name="dscr", bufs=3, space="DRAM"))

        # ---- constants / biases ----
        b1_sb = const.tile([P, KC_C], F32, tag="b1")
        nc.sync.dma_start(b1_sb[:], b1c)
        bq_sb = const.tile([P, MC_G], F32, tag="bq")
        nc.sync.dma_start(bq_sb[:], bqc)
        bk_sb = const.tile([P, MC_G], F32, tag="bk")
        nc.sync.dma_start(bk_sb[:], bkc)
        mb_sb = const.tile([P, SC], F32, tag="mb")
        nc.sync.dma_start(mb_sb[:], mb)
        b2b = const.tile([P, C], F32, tag="b2b")
        nc.gpsimd.dma_start(b2b[:], bcast(b2r, P))
        bvb = const.tile([P, G], F32, tag="bvb")
        nc.gpsimd.dma_start(bvb[:], bcast(bvr, P))
        eps_sb = const.tile([P, 1], F32, tag="eps")
        nc.vector.memset(eps_sb[:], 1e-5)

        # persistent tensors spanning phases
        nT_sb = persist.tile([P, KC_C, S], BF16, tag="nT")
        qT_sb = persist.tile([P, MC_G, T], BF16, tag="qT")
        kT_sb = persist.tile([P, MC_G, S], BF16, tag="kT")
        vAug = persist.tile([P, SC, HL, P], BF16, tag="vAug")
        yT_sb = persist.tile([P, MC_G, T], BF16, tag="yT")
        wp_sb = persist.tile([P, MC_G, C], BF16, tag="wp")
        for kc in range(MC_G):
            nc.sync.dma_start(wp_sb[:, kc, :], wp[kc])

        # ---- phases A+B: encoder MLP + layernorm -> nT ----
        with ExitStack() as ab:
            big = ab.enter_context(tc.tile_pool(name="bigAB", bufs=1))
            zpool = ab.enter_context(tc.tile_pool(name="zpool", bufs=3))
            npool = ab.enter_context(tc.tile_pool(name="npool", bufs=2))
            spool = ab.enter_context(tc.tile_pool(name="spool", bufs=4))

            w1_sb = big.tile([P, KC_E, C], BF16, tag="w1")
            for kc in range(KC_E):
                nc.sync.dma_start(w1_sb[:, kc, :], w1[kc])
            eeT_sb = big.tile([P, KC_E, S], BF16, tag="eeT")
            for kc in range(KC_E):
                nc.sync.dma_start(eeT_sb[:, kc, :], eeT[kc])
            w2_sb = big.tile([P, KC_C, C], BF16, tag="w2")
            for kc in range(KC_C):
                nc.sync.dma_start(w2_sb[:, kc, :], w2[kc])

            # hT = gelu(W1^T @ eeT + b1)  [C, S] bf16
            hT_sb = big.tile([P, KC_C, S], BF16, tag="hT")
            for cc in range(KC_C):
                for n2 in range(S // 512):
                    ps = psA.tile([P, 512], F32, tag="psA")
                    for kc in range(KC_E):
                        nc.tensor.matmul(
                            ps[:],
                            w1_sb[:, kc, ts(cc, P)],
                            eeT_sb[:, kc, ts(n2, 512)],
                            start=(kc == 0),
                            stop=(kc == KC_E - 1),
                        )
                    nc.scalar.activation(
                        hT_sb[:, cc, ts(n2, 512)], ps[:], GELU_FUNC,
                        bias=b1_sb[:, cc : cc + 1],
                    )

            # z = hT^T @ W2 + b2, layernorm -> n (bf16) -> transpose into nT
            for sc in range(SC):
                z_sb = zpool.tile([P, C], F32, tag="z")
                for n2 in range(NC2):
                    ps = psA.tile([P, 512], F32, tag="psA")
                    for kc in range(KC_C):
                        nc.tensor.matmul(
                            ps[:],
                            hT_sb[:, kc, ts(sc, P)],
                            w2_sb[:, kc, ts(n2, 512)],
                            start=(kc == 0),
                            stop=(kc == KC_C - 1),
                        )
                    nc.vector.tensor_tensor(
                        z_sb[:, ts(n2, 512)], ps[:], b2b[:, ts(n2, 512)], ALU.add
                    )
                stats = spool.tile([P, 2, 6], F32, tag="stats")
                nc.vector.bn_stats(stats[:, 0, :], z_sb[:, 0:512])
                nc.vector.bn_stats(stats[:, 1, :], z_sb[:, 512:1024])
                mv = spool.tile([P, 2], F32, tag="mv")
                nc.vector.bn_aggr(mv[:], stats[:])
                sd = spool.tile([P, 1], F32, tag="sd")
                nc.scalar.activation(sd[:], mv[:, 1:2], AF.Sqrt, bias=eps_sb[:])
                rstd = spool.tile([P, 1], F32, tag="rstd")
                nc.vector.reciprocal(rstd[:], sd[:])
                n_sc = npool.tile([P, C], BF16, tag="n")
                nc.vector.tensor_scalar(
                    n_sc[:], z_sb[:], mv[:, 0:1], rstd[:], ALU.subtract, ALU.mult
                )
                nc.sync.dma_start_transpose(nT_sb[:, :, ts(sc, P)], n_sc[:])

        # ---- phase C: qT, kT, vAug ----
        with ExitStack() as pc:
            bigC = pc.enter_context(tc.tile_pool(name="bigC", bufs=1))
            xpool = pc.enter_context(tc.tile_pool(name="xpool", bufs=2))

            wq_sb = bigC.tile([P, KC_C, G], BF16, tag="wq")
            wk_sb = bigC.tile([P, KC_C, G], BF16, tag="wk")
            wv_sb = bigC.tile([P, KC_C, G], BF16, tag="wv")
            for kc in range(KC_C):
                nc.sync.dma_start(wq_sb[:, kc, :], wq[kc])
                nc.sync.dma_start(wk_sb[:, kc, :], wk[kc])
                nc.sync.dma_start(wv_sb[:, kc, :], wv[kc])

            for tc_i in range(NTC):
                xc = xpool.tile([P, KC_C, TCH], BF16, tag="xc")
                for kc in range(KC_C):
                    nc.sync.dma_start(xc[:, kc, :], xT[kc, :, ts(tc_i, TCH)])
                for mc in range(MC_G):
                    ps = psA.tile([P, 512], F32, tag="psA")
                    for kc in range(KC_C):
                        nc.tensor.matmul(
                            ps[:],
                            wq_sb[:, kc, ts(mc, P)],
                            xc[:, kc, :],
                            start=(kc == 0),
                            stop=(kc == KC_C - 1),
                        )
                    nc.scalar.activation(
                        qT_sb[:, mc, ts(tc_i, TCH)], ps[:], AF.Identity,
                        bias=bq_sb[:, mc : mc + 1],
                    )

            for mc in range(MC_G):
                for n2 in range(S // 512):
                    ps = psA.tile([P, 512], F32, tag="psA")
                    for kc in range(KC_C):
                        nc.tensor.matmul(
                            ps[:],
                            wk_sb[:, kc, ts(mc, P)],
                            nT_sb[:, kc, ts(n2, 512)],
                            start=(kc == 0),
                            stop=(kc == KC_C - 1),
                        )
                    nc.scalar.activation(
                        kT_sb[:, mc, ts(n2, 512)], ps[:], AF.Identity,
                        bias=bk_sb[:, mc : mc + 1],
                    )

            # vAug[s, sc, h, :]: even h -> [v(64) | 1 | 0*63]; odd h -> [1 | 0*63 | v(64)]
            # (denominator row must start at a multiple-of-32 partition: 64 / 0)
            nc.vector.memset(vAug[:], 0.0)
            for h in range(HL):
                col = 64 if h % 2 == 0 else 0
                nc.vector.memset(vAug[:, :, h, col : col + 1], 1.0)
            for sc in range(SC):
                ps = psA.tile([P, 512], F32, tag="psA")
                for kc in range(KC_C):
                    nc.tensor.matmul(
                        ps[:],
                        nT_sb[:, kc, ts(sc, P)],
                        wv_sb[:, kc, :],
                        start=(kc == 0),
                        stop=(kc == KC_C - 1),
                    )
                for h in range(HL):
                    off = 0 if h % 2 == 0 else 64
                    nc.vector.tensor_tensor(
                        vAug[:, sc, h, off : off + 64],
                        ps[:, ts(h, 64)],
                        bvb[:, ts(h, 64)],
                        ALU.add,
                    )

        # ---- phase D: attention (+ interleaved partial projection) ----
        with ExitStack() as pd:
            aepool = pd.enter_context(tc.tile_pool(name="aepool", bufs=2))
            rcpool = pd.enter_context(tc.tile_pool(name="rcpool", bufs=2))
            rbpool = pd.enter_context(tc.tile_pool(name="rbpool", bufs=3))
            opool = pd.enter_context(tc.tile_pool(name="opool", bufs=3))

            for tc_i in range(NTC):
                for h in range(HL):
                    hoff = (h % 2) * 64
                    mc = h // 2
                    ae = aepool.tile([P, SC, TCH], BF16, tag="ae")
                    for sc in range(SC):
                        aps = psQK.tile([P, TCH], F32, tag="psQK")
                        nc.tensor.matmul(
                            aps[:],
                            kT_sb[hoff : hoff + 64, mc, ts(sc, P)],
                            qT_sb[hoff : hoff + 64, mc, ts(tc_i, TCH)],
                            start=True,
                            stop=True,
                        )
                        nc.scalar.activation(
                            ae[:, sc, :], aps[:], AF.Exp, bias=mb_sb[:, sc : sc + 1]
                        )
                    yps = psAV.tile([P, TCH], F32, tag="psAV")
                    for sc in range(SC):
                        nc.tensor.matmul(
                            yps[:],
                            vAug[:, sc, h, :],
                            ae[:, sc, :],
                            start=(sc == 0),
                            stop=(sc == SC - 1),
                        )
                    # denominator row: 64 for even h, 0 for odd
                    drow = 64 if h % 2 == 0 else 0
                    yoff = 0 if h % 2 == 0 else 64
                    rc = rcpool.tile([P, TCH], F32, tag="rc")
                    nc.vector.reciprocal(
                        rc[drow : drow + 1, :], yps[drow : drow + 1, :]
                    )
                    scr = dpool.tile([1, TCH], F32, tag="scr")
                    nc.sync.dma_start(scr[:], rc[drow : drow + 1, :])
                    rb = rbpool.tile([P, TCH], F32, tag="rb")
                    s_ap = scr[:]
                    rc_b = bass.AP(
                        tensor=s_ap.tensor,
                        offset=s_ap.offset,
                        ap=[[0, 64]] + [list(x) for x in s_ap.ap[1:]],
                    )
                    nc.gpsimd.dma_start(rb[yoff : yoff + 64, :], rc_b)
                    nc.vector.tensor_tensor(
                        yT_sb[yoff : yoff + 64, mc, ts(tc_i, TCH)],
                        yps[yoff : yoff + 64, :],
                        rb[yoff : yoff + 64, :],
                        ALU.mult,
                    )
                # partial projection for this tc's 4 t-blocks
                for tb in range(tc_i * 4, tc_i * 4 + 4):
                    o_sb = opool.tile([P, C], F32, tag="o")
                    for n2 in range(NC2):
                        ps = psA.tile([P, 512], F32, tag="psA")
                        for kc in range(MC_G):
                            nc.tensor.matmul(
                                ps[:],
                                yT_sb[:, kc, ts(tb, P)],
                                wp_sb[:, kc, ts(n2, 512)],
                                start=(kc == 0),
                                stop=(kc == MC_G - 1),
                            )
                        nc.vector.tensor_copy(o_sb[:, ts(n2, 512)], ps[:])
                    nc.sync.dma_start(out[tb], o_sb[:])

    nc.compile()
    return nc


def make_in_maps(inputs):
    bf16 = ml_dtypes.bfloat16
    f32 = np.float32
    x = np.asarray(inputs["x"], f32)
    ee = np.asarray(inputs["encoder_embedding"], f32)
    mask = np.asarray(inputs["encoder_mask"])
    W1 = np.asarray(inputs["W1"], f32)
    W2 = np.asarray(inputs["W2"], f32)
    b1 = np.asarray(inputs["b1"], f32)
    b2 = np.asarray(inputs["b2"], f32)
    ln_g = np.asarray(inputs["ln_g"], f32)
    ln_b = np.asarray(inputs["ln_b"], f32)
    Wq = np.asarray(inputs["Wq"], f32)
    bq = np.asarray(inputs["bq"], f32)
    Wk = np.asarray(inputs["Wk"], f32)
    bk = np.asarray(inputs["bk"], f32)
    Wv = np.asarray(inputs["Wv"], f32)
    bv = np.asarray(inputs["bv"], f32)
    Wp = np.asarray(inputs["Wp"], f32)

    scale = 1.0 / np.sqrt(np.float32(D))
    Wq_s = Wq * scale
    bq_s = bq * scale
    Wk_f = ln_g[:, None] * Wk
    bk_f = ln_b @ Wk + bk
    Wv_f = ln_g[:, None] * Wv
    bv_f = ln_b @ Wv + bv

    in_maps = []
    for c in range(8):
        b = c // 2
        g = c % 2
        gs = slice(g * G, (g + 1) * G)
        m = {
            "eeT": np.ascontiguousarray(ee[b].T).reshape(KC_E, P, S).astype(bf16),
            "xT": np.ascontiguousarray(x[b].T).reshape(KC_C, P, T).astype(bf16),
            "w1": W1.reshape(KC_E, P, C).astype(bf16),
            "w2": W2.reshape(KC_C, P, C).astype(bf16),
            "wq": Wq_s[:, gs].reshape(KC_C, P, G).astype(bf16),
            "wk": Wk_f[:, gs].reshape(KC_C, P, G).astype(bf16),
            "wv": Wv_f[:, gs].reshape(KC_C, P, G).astype(bf16),
            "wp": Wp[gs, :].reshape(MC_G, P, C).astype(bf16),
            "b1c": np.ascontiguousarray(b1.reshape(KC_C, P).T),
            "b2r": b2.reshape(1, C),
            "bqc": np.ascontiguousarray(bq_s[gs].reshape(MC_G, P).T),
            "bkc": np.ascontiguousarray(bk_f[gs].reshape(MC_G, P).T),
            "bvr": bv_f[gs].reshape(1, G),
            "mb": np.ascontiguousarray(
                ((mask[b].astype(f32) - 1.0) * 30.0).reshape(SC, P).T
            ),
        }
        in_maps.append(m)
    return in_maps


_NC_CACHE = None


def _get_nc():
    global _NC_CACHE
    if _NC_CACHE is None:
        _NC_CACHE = build_program()
    return _NC_CACHE


def kernel(**inputs):
    from concourse.bass_utils import run_bass_kernel_spmd

    nc = _get_nc()
    in_maps = make_in_maps(inputs)
    res = run_bass_kernel_spmd(nc, in_maps, core_ids=list(range(8)))
    bp = np.asarray(inputs["bp"], np.float32)
    outs = [res.results[c]["out"].reshape(T, C) for c in range(8)]
    y = np.stack([outs[2 * b] + outs[2 * b + 1] + bp for b in range(B)])
    return y.astype(np.float32)


# revision 9
# speedup vs baseline: 1.8544x; 1.8544x over previous
"""Trainium2 Bass kernel for CrossAttention (B=4, T=2048, S=1024, C=1024, H=16).

Sharding: 8 cores = batch (4) x head-group (2 groups of 8 heads).
Each core computes the encoder MLP+LN for its batch (duplicated across the
2 head-group cores), its head-group's q/k/v, attention, and a partial
output projection. Host sums the two partial projections per batch + bp.

All matmuls run in bf16 (fp32 PSUM accumulation). Dataflow is transposed
(qT/kT/yT layouts) so no on-device transposes are needed except one bf16
DMA-transpose of the normalized encoder activations. Softmax denominators
come from a ones-column folded into the AV matmul; the attention mask and
1/sqrt(D) scale are folded into the exp bias / q-projection weights.
"""

import sys

sys.path.insert(0, "/opt/trn_rl_repo")

import numpy as np
import ml_dtypes
from contextlib import ExitStack

import concourse.bass as bass
import concourse.tile as tile
from concourse import bacc, mybir
from concourse.bass import ts

F32 = mybir.dt.float32
BF16 = mybir.dt.bfloat16
AF = mybir.ActivationFunctionType
ALU = mybir.AluOpType

# CoreSim doesn't implement the Gelu table; tests swap this to Identity to
# validate dataflow in sim. HW always uses the real erf-based Gelu.
GELU_FUNC = AF.Gelu

# Problem dims (fixed by the reference)
B, T, S, C, E, H, D = 4, 2048, 1024, 1024, 512, 16, 64
G = C // 2  # head-group width (8 heads * 64)
HL = G // D  # 8 local heads
P = 128
TCH = 512  # attention t-chunk
KC_C = C // P  # 8
KC_E = E // P  # 4
MC_G = G // P  # 4
SC = S // P  # 8
NTC = T // TCH  # 4
TB = T // P  # 16
NC2 = C // 512  # 2


def build_program(reps=1):
    """reps>1 repeats the whole body back-to-back (for marginal timing)."""
    nc = bacc.Bacc("TRN2", target_bir_lowering=False, debug=False)

    def din(name, shape, dtype=BF16):
        return nc.dram_tensor(name, shape, dtype, kind="ExternalInput").ap()

    eeT = din("eeT", [KC_E, P, S])
    xT = din("xT", [KC_C, P, T])
    w1 = din("w1", [KC_E, P, C])
    w2 = din("w2", [KC_C, P, C])
    wq = din("wq", [KC_C, P, G])
    wk = din("wk", [KC_C, P, G])
    wv = din("wv", [KC_C, P, G])
    wp = din("wp", [MC_G, P, C])
    b1c = din("b1c", [P, KC_C], F32)
    b2r = din("b2r", [1, C], F32)
    bqc = din("bqc", [P, MC_G], F32)
    bkc = din("bkc", [P, MC_G], F32)
    bvr = din("bvr", [1, G], F32)
    mb = din("mb", [P, SC], F32)
    out = nc.dram_tensor("out", [TB, P, C], F32, kind="ExternalOutput").ap()

    def bcast(src_row, parts):
        # DRAM [1, N] row -> partition-broadcast AP [parts, N]
        return bass.AP(
            tensor=src_row.tensor,
            offset=src_row.offset,
            ap=[[0, parts]] + [list(x) for x in src_row.ap[1:]],
        )

    with tile.TileContext(nc) as tc, ExitStack() as _ctx0:
      for _rep in range(reps):
        ctx = ExitStack()
        const = ctx.enter_context(tc.tile_pool(name="const", bufs=1))
        persist = ctx.enter_context(tc.tile_pool(name="persist", bufs=1))
        psA = ctx.enter_context(tc.tile_pool(name="psA", bufs=3, space="PSUM"))
        psQK = ctx.enter_context(tc.tile_pool(name="psQK", bufs=3, space="PSUM"))
        psAV = ctx.enter_context(tc.tile_pool(name="psAV", bufs=2, space="PSUM"))
        dpool = ctx.enter_context(tc.tile_pool(name="dscr", bufs=3, space="DRAM"))

        # ---- constants / biases ----
        b1_sb = const.tile([P, KC_C], F32, tag="b1")
        nc.sync.dma_start(b1_sb[:], b1c)
        bq_sb = const.tile([P, MC_G], F32, tag="bq")
        nc.sync.dma_start(bq_sb[:], bqc)
        bk_sb = const.tile([P, MC_G], F32, tag="bk")
        nc.sync.dma_start(bk_sb[:], bkc)
        mb_sb = const.tile([P, SC], F32, tag="mb")
        nc.sync.dma_start(mb_sb[:], mb)
        b2b = const.tile([P, C], F32, tag="b2b")
        nc.gpsimd.dma_start(b2b[:], bcast(b2r, P))
        bvb = const.tile([P, G], F32, tag="bvb")
        nc.gpsimd.dma_start(bvb[:], bcast(bvr, P))
        eps_sb = const.tile([P, 1], F32, tag="eps")
        nc.vector.memset(eps_sb[:], 1e-5)

        # persistent tensors spanning phases
        nT_sb = persist.tile([P, KC_C, S], BF16, tag="nT")
        qT_sb = persist.tile([P, MC_G, T], BF16, tag="qT")
        kT_sb = persist.tile([P, MC_G, S], BF16, tag="kT")
        vAug = persist.tile([P, SC, HL, P], BF16, tag="vAug")
        yT_sb = persist.tile([P, MC_G, T], BF16, tag="yT")
        wp_sb = persist.tile([P, MC_G, C], BF16, tag="wp")
        for kc in range(MC_G):
            nc.sync.dma_start(wp_sb[:, kc, :], wp[kc])

        # ---- phases A+B: encoder MLP + layernorm -> nT ----
        with ExitStack() as ab:
            big = ab.enter_context(tc.tile_pool(name="bigAB", bufs=1))
            zpool = ab.enter_context(tc.tile_pool(name="zpool", bufs=3))
            npool = ab.enter_context(tc.tile_pool(name="npool", bufs=2))
            spool = ab.enter_context(tc.tile_pool(name="spool", bufs=4))

            w1_sb = big.tile([P, KC_E, C], BF16, tag="w1")
            for kc in range(KC_E):
                nc.sync.dma_start(w1_sb[:, kc, :], w1[kc])
            eeT_sb = big.tile([P, KC_E, S], BF16, tag="eeT")
            for kc in range(KC_E):
                nc.sync.dma_start(eeT_sb[:, kc, :], eeT[kc])
            w2_sb = big.tile([P, KC_C, C], BF16, tag="w2")
            for kc in range(KC_C):
                nc.sync.dma_start(w2_sb[:, kc, :], w2[kc])

            # hT = gelu(W1^T @ eeT + b1)  [C, S] bf16
            hT_sb = big.tile([P, KC_C, S], BF16, tag="hT")
            for cc in range(KC_C):
                for n2 in range(S // 512):
                    ps = psA.tile([P, 512], F32, tag="psA")
                    for kc in range(KC_E):
                        nc.tensor.matmul(
                            ps[:],
                            w1_sb[:, kc, ts(cc, P)],
                            eeT_sb[:, kc, ts(n2, 512)],
                            start=(kc == 0),
                            stop=(kc == KC_E - 1),
                        )
                    nc.scalar.activation(
                        hT_sb[:, cc, ts(n2, 512)], ps[:], GELU_FUNC,
                        bias=b1_sb[:, cc : cc + 1],
                    )

            # z = hT^T @ W2 + b2, layernorm -> n (bf16) -> transpose into nT
            for sc in range(SC):
                z_sb = zpool.tile([P, C], F32, tag="z")
                for n2 in range(NC2):
                    ps = psA.tile([P, 512], F32, tag="psA")
                    for kc in range(KC_C):
                        nc.tensor.matmul(
                            ps[:],
                            hT_sb[:, kc, ts(sc, P)],
                            w2_sb[:, kc, ts(n2, 512)],
                            start=(kc == 0),
                            stop=(kc == KC_C - 1),
                        )
                    nc.vector.tensor_tensor(
                        z_sb[:, ts(n2, 512)], ps[:], b2b[:, ts(n2, 512)], ALU.add
                    )
                stats = spool.tile([P, 2, 6], F32, tag="stats")
                nc.vector.bn_stats(stats[:, 0, :], z_sb[:, 0:512])
                nc.vector.bn_stats(stats[:, 1, :], z_sb[:, 512:1024])
                mv = spool.tile([P, 2], F32, tag="mv")
                nc.vector.bn_aggr(mv[:], stats[:])
                sd = spool.tile([P, 1], F32, tag="sd")
                nc.scalar.activation(sd[:], mv[:, 1:2], AF.Sqrt, bias=eps_sb[:])
                rstd = spool.tile([P, 1], F32, tag="rstd")
                nc.vector.reciprocal(rstd[:], sd[:])
                n_sc = npool.tile([P, C], BF16, tag="n")
                nc.vector.tensor_scalar(
                    n_sc[:], z_sb[:], mv[:, 0:1], rstd[:], ALU.subtract, ALU.mult
                )
                nc.sync.dma_start_transpose(nT_sb[:, :, ts(sc, P)], n_sc[:])

        # ---- phase C: qT, kT, vAug ----
        with ExitStack() as pc:
            bigC = pc.enter_context(tc.tile_pool(name="bigC", bufs=1))
            xpool = pc.enter_context(tc.tile_pool(name="xpool", bufs=2))

            wq_sb = bigC.tile([P, KC_C, G], BF16, tag="wq")
            wk_sb = bigC.tile([P, KC_C, G], BF16, tag="wk")
            wv_sb = bigC.tile([P, KC_C, G], BF16, tag="wv")
            for kc in range(KC_C):
                nc.sync.dma_start(wq_sb[:, kc, :], wq[kc])
                nc.sync.dma_start(wk_sb[:, kc, :], wk[kc])
                nc.sync.dma_start(wv_sb[:, kc, :], wv[kc])

            for tc_i in range(NTC):
                xc = xpool.tile([P, KC_C, TCH], BF16, tag="xc")
                for kc in range(KC_C):
                    nc.sync.dma_start(xc[:, kc, :], xT[kc, :, ts(tc_i, TCH)])
                for mc in range(MC_G):
                    ps = psA.tile([P, 512], F32, tag="psA")
                    for kc in range(KC_C):
                        nc.tensor.matmul(
                            ps[:],
                            wq_sb[:, kc, ts(mc, P)],
                            xc[:, kc, :],
                            start=(kc == 0),
                            stop=(kc == KC_C - 1),
                        )
                    nc.scalar.activation(
                        qT_sb[:, mc, ts(tc_i, TCH)], ps[:], AF.Identity,
                        bias=bq_sb[:, mc : mc + 1],
                    )

            for mc in range(MC_G):
                for n2 in range(S // 512):
                    ps = psA.tile([P, 512], F32, tag="psA")
                    for kc in range(KC_C):
                        nc.tensor.matmul(
                            ps[:],
                            wk_sb[:, kc, ts(mc, P)],
                            nT_sb[:, kc, ts(n2, 512)],
                            start=(kc == 0),
                            stop=(kc == KC_C - 1),
                        )
                    nc.scalar.activation(
                        kT_sb[:, mc, ts(n2, 512)], ps[:], AF.Identity,
                        bias=bk_sb[:, mc : mc + 1],
                    )

            # vAug[s, sc, h, :]: even h -> [v(64) | 1 | 0*63]; odd h -> [1 | 0*63 | v(64)]
            # (denominator row must start at a multiple-of-32 partition: 64 / 0)
            nc.vector.memset(vAug[:], 0.0)
            for h in range(HL):
                col = 64 if h % 2 == 0 else 0
                nc.vector.memset(vAug[:, :, h, col : col + 1], 1.0)
            for sc in range(SC):
                ps = psA.tile([P, 512], F32, tag="psA")
                for kc in range(KC_C):
                    nc.tensor.matmul(
                        ps[:],
                        nT_sb[:, kc, ts(sc, P)],
                        wv_sb[:, kc, :],
                        start=(kc == 0),
                        stop=(kc == KC_C - 1),
                    )
                for h in range(HL):
                    off = 0 if h % 2 == 0 else 64
                    nc.vector.tensor_tensor(
                        vAug[:, sc, h, off : off + 64],
                        ps[:, ts(h, 64)],
                        bvb[:, ts(h, 64)],
                        ALU.add,
                    )

        # ---- phase D: attention (+ interleaved partial projection) ----
        with ExitStack() as pd:
            aepool = pd.enter_context(tc.tile_pool(name="aepool", bufs=2))
            rcpool = pd.enter_context(tc.tile_pool(name="rcpool", bufs=2))
            rbpool = pd.enter_context(tc.tile_pool(name="rbpool", bufs=3))
            opool = pd.enter_context(tc.tile_pool(name="opool", bufs=3))

            for tc_i in range(NTC):
                for h in range(HL):
                    hoff = (h % 2) * 64
                    mc = h // 2
                    ae = aepool.tile([P, SC, TCH], BF16, tag="ae")
                    for sc in range(SC):
                        aps = psQK.tile([P, TCH], F32, tag="psQK")
                        nc.tensor.matmul(
                            aps[:],
                            kT_sb[hoff : hoff + 64, mc, ts(sc, P)],
                            qT_sb[hoff : hoff + 64, mc, ts(tc_i, TCH)],
                            start=True,
                            stop=True,
                        )
                        nc.scalar.activation(
                            ae[:, sc, :], aps[:], AF.Exp, bias=mb_sb[:, sc : sc + 1]
                        )
                    yps = psAV.tile([P, TCH], F32, tag="psAV")
                    for sc in range(SC):
                        nc.tensor.matmul(
                            yps[:],
                            vAug[:, sc, h, :],
                            ae[:, sc, :],
                            start=(sc == 0),
                            stop=(sc == SC - 1),
                        )
                    # denominator row: 64 for even h, 0 for odd
                    drow = 64 if h % 2 == 0 else 0
                    yoff = 0 if h % 2 == 0 else 64
                    rc = rcpool.tile([P, TCH], F32, tag="rc")
                    nc.vector.reciprocal(
                        rc[drow : drow + 1, :], yps[drow : drow + 1, :]
                    )
                    scr = dpool.tile([1, TCH], F32, tag="scr")
                    nc.sync.dma_start(scr[:], rc[drow : drow + 1, :])
                    rb = rbpool.tile([P, TCH], F32, tag="rb")
                    s_ap = scr[:]
                    rc_b = bass.AP(
                        tensor=s_ap.tensor,
                        offset=s_ap.offset,
                        ap=[[0, 64]] + [list(x) for x in s_ap.ap[1:]],
                    )
                    nc.gpsimd.dma_start(rb[yoff : yoff + 64, :], rc_b)
                    nc.vector.tensor_tensor(
                        yT_sb[yoff : yoff + 64, mc, ts(tc_i, TCH)],
                        yps[yoff : yoff + 64, :],
                        rb[yoff : yoff + 64, :],
                        ALU.mult,
                    )
                # partial projection for this tc's 4 t-blocks
                for tb in range(tc_i * 4, tc_i * 4 + 4):
                    o_sb = opool.tile([P, C], F32, tag="o")
                    for n2 in range(NC2):
                        ps = psA.tile([P, 512], F32, tag="psA")
                        for kc in range(MC_G):
                            nc.tensor.matmul(
                                ps[:],
                                yT_sb[:, kc, ts(tb, P)],
                                wp_sb[:, kc, ts(n2, 512)],
                                start=(kc == 0),
                                stop=(kc == MC_G - 1),
                            )
                        nc.vector.tensor_copy(o_sb[:, ts(n2, 512)], ps[:])
                    nc.sync.dma_start(out[tb], o_sb[:])

        ctx.close()

    nc.compile()
    return nc


def make_in_maps(inputs):
    bf16 = ml_dtypes.bfloat16
    f32 = np.float32
    x = np.asarray(inputs["x"], f32)
    ee = np.asarray(inputs["encoder_embedding"], f32)
    mask = np.asarray(inputs["encoder_mask"])
    W1 = np.asarray(inputs["W1"], f32)
    W2 = np.asarray(inputs["W2"], f32)
    b1 = np.asarray(inputs["b1"], f32)
    b2 = np.asarray(inputs["b2"], f32)
    ln_g = np.asarray(inputs["ln_g"], f32)
    ln_b = np.asarray(inputs["ln_b"], f32)
    Wq = np.asarray(inputs["Wq"], f32)
    bq = np.asarray(inputs["bq"], f32)
    Wk = np.asarray(inputs["Wk"], f32)
    bk = np.asarray(inputs["bk"], f32)
    Wv = np.asarray(inputs["Wv"], f32)
    bv = np.asarray(inputs["bv"], f32)
    Wp = np.asarray(inputs["Wp"], f32)

    scale = 1.0 / np.sqrt(np.float32(D))
    Wq_s = Wq * scale
    bq_s = bq * scale
    Wk_f = ln_g[:, None] * Wk
    bk_f = ln_b @ Wk + bk
    Wv_f = ln_g[:, None] * Wv
    bv_f = ln_b @ Wv + bv

    in_maps = []
    for c in range(8):
        b = c // 2
        g = c % 2
        gs = slice(g * G, (g + 1) * G)
        m = {
            "eeT": np.ascontiguousarray(ee[b].T).reshape(KC_E, P, S).astype(bf16),
            "xT": np.ascontiguousarray(x[b].T).reshape(KC_C, P, T).astype(bf16),
            "w1": W1.reshape(KC_E, P, C).astype(bf16),
            "w2": W2.reshape(KC_C, P, C).astype(bf16),
            "wq": Wq_s[:, gs].reshape(KC_C, P, G).astype(bf16),
            "wk": Wk_f[:, gs].reshape(KC_C, P, G).astype(bf16),
            "wv": Wv_f[:, gs].reshape(KC_C, P, G).astype(bf16),
            "wp": Wp[gs, :].reshape(MC_G, P, C).astype(bf16),
            "b1c": np.ascontiguousarray(b1.reshape(KC_C, P).T),
            "b2r": b2.reshape(1, C),
            "bqc": np.ascontiguousarray(bq_s[gs].reshape(MC_G, P).T),
            "bkc": np.ascontiguousarray(bk_f[gs].reshape(MC_G, P).T),
            "bvr": bv_f[gs].reshape(1, G),
            "mb": np.ascontiguousarray(
                ((mask[b].astype(f32) - 1.0) * 30.0).reshape(SC, P).T
            ),
        }
        in_maps.append(m)
    return in_maps


_NC_CACHE = None


def _get_nc():
    global _NC_CACHE
    if _NC_CACHE is None:
        _NC_CACHE = build_program()
    return _NC_CACHE


def kernel(**inputs):
    from concourse.bass_utils import run_bass_kernel_spmd

    nc = _get_nc()
    in_maps = make_in_maps(inputs)
    res = run_bass_kernel_spmd(nc, in_maps, core_ids=list(range(8)))
    bp = np.asarray(inputs["bp"], np.float32)
    outs = [res.results[c]["out"].reshape(T, C) for c in range(8)]
    y = np.stack([outs[2 * b] + outs[2 * b + 1] + bp for b in range(B)])
    return y.astype(np.float32)


# revision 11
# speedup vs baseline: 3.0670x; 1.6539x over previous
"""Trainium2 Bass kernel for CrossAttention (B=4, T=2048, S=1024, C=1024, H=16).

Sharding: 8 cores = batch (4) x head-group (2 groups of 8 heads).
Each core computes the encoder MLP+LN for its batch (duplicated across the
2 head-group cores), its head-group's q/k/v, attention, and a partial
output projection. Host sums the two partial projections per batch + bp.

All matmuls run in bf16 (fp32 PSUM accumulation). Dataflow is transposed
(qT/kT/yT layouts) so no on-device transposes are needed except one bf16
DMA-transpose of the normalized encoder activations. Softmax denominators
come from a ones-column folded into the AV matmul; the attention mask and
1/sqrt(D) scale are folded into the exp bias / q-projection weights.
"""

import sys

sys.path.insert(0, "/opt/trn_rl_repo")

import numpy as np
import ml_dtypes
from contextlib import ExitStack

import concourse.bass as bass
import concourse.tile as tile
from concourse import bacc, mybir
from concourse.bass import ts

F32 = mybir.dt.float32
BF16 = mybir.dt.bfloat16
AF = mybir.ActivationFunctionType
ALU = mybir.AluOpType

# CoreSim doesn't implement the Gelu table; tests swap this to Identity to
# validate dataflow in sim. HW always uses the real erf-based Gelu.
GELU_FUNC = AF.Gelu

# Problem dims (fixed by the reference)
B, T, S, C, E, H, D = 4, 2048, 1024, 1024, 512, 16, 64
G = C // 2  # head-group width (8 heads * 64)
HL = G // D  # 8 local heads
P = 128
TCH = 512  # attention t-chunk
KC_C = C // P  # 8
KC_E = E // P  # 4
MC_G = G // P  # 4
SC = S // P  # 8
NTC = T // TCH  # 4
TB = T // P  # 16
NC2 = C // 512  # 2


def build_program(reps=1):
    """reps>1 repeats the whole body back-to-back (for marginal timing)."""
    nc = bacc.Bacc("TRN2", target_bir_lowering=False, debug=False)

    def din(name, shape, dtype=BF16):
        return nc.dram_tensor(name, shape, dtype, kind="ExternalInput").ap()

    eeT = din("eeT", [KC_E, P, S])
    xT = din("xT", [KC_C, P, T])
    w1 = din("w1", [KC_E, P, C])
    w2 = din("w2", [KC_C, P, C])
    wq = din("wq", [KC_C, P, G])
    wk = din("wk", [KC_C, P, G])
    wv = din("wv", [KC_C, P, G])
    wp = din("wp", [MC_G, P, C])
    b1c = din("b1c", [P, KC_C], F32)
    b2r = din("b2r", [1, C], F32)
    bqc = din("bqc", [P, MC_G], F32)
    bkc = din("bkc", [P, MC_G], F32)
    bvr = din("bvr", [1, G], F32)
    mb = din("mb", [P, SC], F32)
    out = nc.dram_tensor("out", [TB, P, C], F32, kind="ExternalOutput").ap()

    def bcast(src_row, parts):
        # DRAM [1, N] row -> partition-broadcast AP [parts, N]
        return bass.AP(
            tensor=src_row.tensor,
            offset=src_row.offset,
            ap=[[0, parts]] + [list(x) for x in src_row.ap[1:]],
        )

    with tile.TileContext(nc) as tc, ExitStack() as _ctx0:
      for _rep in range(reps):
        ctx = ExitStack()
        const = ctx.enter_context(tc.tile_pool(name="const", bufs=1))
        persist = ctx.enter_context(tc.tile_pool(name="persist", bufs=1))
        psA = ctx.enter_context(tc.tile_pool(name="psA", bufs=3, space="PSUM"))
        psQK = ctx.enter_context(tc.tile_pool(name="psQK", bufs=3, space="PSUM"))
        psAV = ctx.enter_context(tc.tile_pool(name="psAV", bufs=2, space="PSUM"))
        dpool = ctx.enter_context(tc.tile_pool(name="dscr", bufs=3, space="DRAM"))

        # ---- constants / biases ----
        b1_sb = const.tile([P, KC_C], F32, tag="b1")
        nc.sync.dma_start(b1_sb[:], b1c)
        bq_sb = const.tile([P, MC_G], F32, tag="bq")
        nc.sync.dma_start(bq_sb[:], bqc)
        bk_sb = const.tile([P, MC_G], F32, tag="bk")
        nc.sync.dma_start(bk_sb[:], bkc)
        mb_sb = const.tile([P, SC], F32, tag="mb")
        nc.sync.dma_start(mb_sb[:], mb)
        b2b = const.tile([P, C], F32, tag="b2b")
        nc.gpsimd.dma_start(b2b[:], bcast(b2r, P))
        bvb = const.tile([P, G], F32, tag="bvb")
        nc.gpsimd.dma_start(bvb[:], bcast(bvr, P))
        eps_sb = const.tile([P, 1], F32, tag="eps")
        nc.vector.memset(eps_sb[:], 1e-5)

        # persistent tensors spanning phases
        nT_sb = persist.tile([P, KC_C, S], BF16, tag="nT")
        qT_sb = persist.tile([P, MC_G, T], BF16, tag="qT")
        kT_sb = persist.tile([P, MC_G, S], BF16, tag="kT")
        vAug = persist.tile([P, SC, HL, P], BF16, tag="vAug")
        yT_sb = persist.tile([P, MC_G, T], BF16, tag="yT")
        wp_sb = persist.tile([P, MC_G, C], BF16, tag="wp")
        for kc in range(MC_G):
            nc.sync.dma_start(wp_sb[:, kc, :], wp[kc])

        # ---- phases A+B: encoder MLP + layernorm -> nT ----
        with ExitStack() as ab:
            big = ab.enter_context(tc.tile_pool(name="bigAB", bufs=1))
            zpool = ab.enter_context(tc.tile_pool(name="zpool", bufs=3))
            npool = ab.enter_context(tc.tile_pool(name="npool", bufs=2))
            spool = ab.enter_context(tc.tile_pool(name="spool", bufs=4))

            w1_sb = big.tile([P, KC_E, C], BF16, tag="w1")
            for kc in range(KC_E):
                nc.sync.dma_start(w1_sb[:, kc, :], w1[kc])
            eeT_sb = big.tile([P, KC_E, S], BF16, tag="eeT")
            for kc in range(KC_E):
                nc.sync.dma_start(eeT_sb[:, kc, :], eeT[kc])
            w2_sb = big.tile([P, KC_C, C], BF16, tag="w2")
            for kc in range(KC_C):
                nc.sync.dma_start(w2_sb[:, kc, :], w2[kc])

            # hT = gelu(W1^T @ eeT + b1)  [C, S] bf16
            hT_sb = big.tile([P, KC_C, S], BF16, tag="hT")
            for cc in range(KC_C):
                for n2 in range(S // 512):
                    ps = psA.tile([P, 512], F32, tag="psA")
                    for kc in range(KC_E):
                        nc.tensor.matmul(
                            ps[:],
                            w1_sb[:, kc, ts(cc, P)],
                            eeT_sb[:, kc, ts(n2, 512)],
                            start=(kc == 0),
                            stop=(kc == KC_E - 1),
                        )
                    nc.scalar.activation(
                        hT_sb[:, cc, ts(n2, 512)], ps[:], GELU_FUNC,
                        bias=b1_sb[:, cc : cc + 1],
                    )

            # z = hT^T @ W2 + b2, layernorm -> n (bf16) -> transpose into nT
            for sc in range(SC):
                z_sb = zpool.tile([P, C], F32, tag="z")
                for n2 in range(NC2):
                    ps = psA.tile([P, 512], F32, tag="psA")
                    for kc in range(KC_C):
                        nc.tensor.matmul(
                            ps[:],
                            hT_sb[:, kc, ts(sc, P)],
                            w2_sb[:, kc, ts(n2, 512)],
                            start=(kc == 0),
                            stop=(kc == KC_C - 1),
                        )
                    nc.vector.tensor_tensor(
                        z_sb[:, ts(n2, 512)], ps[:], b2b[:, ts(n2, 512)], ALU.add
                    )
                stats = spool.tile([P, 2, 6], F32, tag="stats")
                nc.vector.bn_stats(stats[:, 0, :], z_sb[:, 0:512])
                nc.vector.bn_stats(stats[:, 1, :], z_sb[:, 512:1024])
                mv = spool.tile([P, 2], F32, tag="mv")
                nc.vector.bn_aggr(mv[:], stats[:])
                sd = spool.tile([P, 1], F32, tag="sd")
                nc.scalar.activation(sd[:], mv[:, 1:2], AF.Sqrt, bias=eps_sb[:])
                rstd = spool.tile([P, 1], F32, tag="rstd")
                nc.vector.reciprocal(rstd[:], sd[:])
                n_sc = npool.tile([P, C], BF16, tag="n")
                nc.vector.tensor_scalar(
                    n_sc[:], z_sb[:], mv[:, 0:1], rstd[:], ALU.subtract, ALU.mult
                )
                nc.sync.dma_start_transpose(nT_sb[:, :, ts(sc, P)], n_sc[:])

        # ---- phase C: qT, kT, vAug ----
        with ExitStack() as pc:
            bigC = pc.enter_context(tc.tile_pool(name="bigC", bufs=1))
            xpool = pc.enter_context(tc.tile_pool(name="xpool", bufs=2))

            wq_sb = bigC.tile([P, KC_C, G], BF16, tag="wq")
            wk_sb = bigC.tile([P, KC_C, G], BF16, tag="wk")
            wv_sb = bigC.tile([P, KC_C, G], BF16, tag="wv")
            for kc in range(KC_C):
                nc.sync.dma_start(wq_sb[:, kc, :], wq[kc])
                nc.sync.dma_start(wk_sb[:, kc, :], wk[kc])
                nc.sync.dma_start(wv_sb[:, kc, :], wv[kc])

            for tc_i in range(NTC):
                xc = xpool.tile([P, KC_C, TCH], BF16, tag="xc")
                for kc in range(KC_C):
                    nc.sync.dma_start(xc[:, kc, :], xT[kc, :, ts(tc_i, TCH)])
                for mc in range(MC_G):
                    ps = psA.tile([P, 512], F32, tag="psA")
                    for kc in range(KC_C):
                        nc.tensor.matmul(
                            ps[:],
                            wq_sb[:, kc, ts(mc, P)],
                            xc[:, kc, :],
                            start=(kc == 0),
                            stop=(kc == KC_C - 1),
                        )
                    nc.scalar.activation(
                        qT_sb[:, mc, ts(tc_i, TCH)], ps[:], AF.Identity,
                        bias=bq_sb[:, mc : mc + 1],
                    )

            for mc in range(MC_G):
                for n2 in range(S // 512):
                    ps = psA.tile([P, 512], F32, tag="psA")
                    for kc in range(KC_C):
                        nc.tensor.matmul(
                            ps[:],
                            wk_sb[:, kc, ts(mc, P)],
                            nT_sb[:, kc, ts(n2, 512)],
                            start=(kc == 0),
                            stop=(kc == KC_C - 1),
                        )
                    nc.scalar.activation(
                        kT_sb[:, mc, ts(n2, 512)], ps[:], AF.Identity,
                        bias=bk_sb[:, mc : mc + 1],
                    )

            # vAug[s, sc, h, :]: even h -> [v(64) | 1 | 0*63]; odd h -> [1 | 0*63 | v(64)]
            # (denominator row must start at a multiple-of-32 partition: 64 / 0)
            nc.vector.memset(vAug[:], 0.0)
            for h in range(HL):
                col = 64 if h % 2 == 0 else 0
                nc.vector.memset(vAug[:, :, h, col : col + 1], 1.0)
            for sc in range(SC):
                ps = psA.tile([P, 512], F32, tag="psA")
                for kc in range(KC_C):
                    nc.tensor.matmul(
                        ps[:],
                        nT_sb[:, kc, ts(sc, P)],
                        wv_sb[:, kc, :],
                        start=(kc == 0),
                        stop=(kc == KC_C - 1),
                    )
                for h in range(HL):
                    off = 0 if h % 2 == 0 else 64
                    nc.vector.tensor_tensor(
                        vAug[:, sc, h, off : off + 64],
                        ps[:, ts(h, 64)],
                        bvb[:, ts(h, 64)],
                        ALU.add,
                    )

        # ---- phase D: attention (+ interleaved partial projection) ----
        with ExitStack() as pd:
            aepool = pd.enter_context(tc.tile_pool(name="aepool", bufs=2))
            rcpool = pd.enter_context(tc.tile_pool(name="rcpool", bufs=2))
            rbpool = pd.enter_context(tc.tile_pool(name="rbpool", bufs=3))
            opool = pd.enter_context(tc.tile_pool(name="opool", bufs=3))

            for tc_i in range(NTC):
                for h in range(HL):
                    hoff = (h % 2) * 64
                    mc = h // 2
                    ae = aepool.tile([P, SC, TCH], BF16, tag="ae")
                    for sc in range(SC):
                        aps = psQK.tile([P, TCH], F32, tag="psQK")
                        nc.tensor.matmul(
                            aps[:],
                            kT_sb[hoff : hoff + 64, mc, ts(sc, P)],
                            qT_sb[hoff : hoff + 64, mc, ts(tc_i, TCH)],
                            start=True,
                            stop=True,
                        )
                        nc.scalar.activation(
                            ae[:, sc, :], aps[:], AF.Exp, bias=mb_sb[:, sc : sc + 1]
                        )
                    yps = psAV.tile([P, TCH], F32, tag="psAV")
                    for sc in range(SC):
                        nc.tensor.matmul(
                            yps[:],
                            vAug[:, sc, h, :],
                            ae[:, sc, :],
                            start=(sc == 0),
                            stop=(sc == SC - 1),
                        )
                    # denominator row: 64 for even h, 0 for odd
                    drow = 64 if h % 2 == 0 else 0
                    yoff = 0 if h % 2 == 0 else 64
                    # evict psum early (frees the AV bank; the recip/broadcast
                    # DMA chain then runs off SBUF without stalling PE)
                    ycp = rcpool.tile([P, TCH], F32, tag="ycp")
                    nc.vector.tensor_copy(
                        ycp[yoff : yoff + 64, :], yps[yoff : yoff + 64, :]
                    )
                    rc = rcpool.tile([P, TCH], F32, tag="rc")
                    nc.vector.reciprocal(
                        rc[drow : drow + 1, :], yps[drow : drow + 1, :]
                    )
                    scr = dpool.tile([1, TCH], F32, tag="scr")
                    nc.sync.dma_start(scr[:], rc[drow : drow + 1, :])
                    rb = rbpool.tile([P, TCH], F32, tag="rb")
                    s_ap = scr[:]
                    rc_b = bass.AP(
                        tensor=s_ap.tensor,
                        offset=s_ap.offset,
                        ap=[[0, 64]] + [list(x) for x in s_ap.ap[1:]],
                    )
                    nc.gpsimd.dma_start(rb[yoff : yoff + 64, :], rc_b)
                    nc.vector.tensor_tensor(
                        yT_sb[yoff : yoff + 64, mc, ts(tc_i, TCH)],
                        ycp[yoff : yoff + 64, :],
                        rb[yoff : yoff + 64, :],
                        ALU.mult,
                    )
                # partial projection for this tc's 4 t-blocks
                for tb in range(tc_i * 4, tc_i * 4 + 4):
                    o_sb = opool.tile([P, C], F32, tag="o")
                    for n2 in range(NC2):
                        ps = psA.tile([P, 512], F32, tag="psA")
                        for kc in range(MC_G):
                            nc.tensor.matmul(
                                ps[:],
                                yT_sb[:, kc, ts(tb, P)],
                                wp_sb[:, kc, ts(n2, 512)],
                                start=(kc == 0),
                                stop=(kc == MC_G - 1),
                            )
                        nc.vector.tensor_copy(o_sb[:, ts(n2, 512)], ps[:])
                    nc.sync.dma_start(out[tb], o_sb[:])

        ctx.close()

    nc.compile()
    return nc


def make_in_maps(inputs):
    bf16 = ml_dtypes.bfloat16
    f32 = np.float32
    x = np.asarray(inputs["x"], f32)
    ee = np.asarray(inputs["encoder_embedding"], f32)
    mask = np.asarray(inputs["encoder_mask"])
    W1 = np.asarray(inputs["W1"], f32)
    W2 = np.asarray(inputs["W2"], f32)
    b1 = np.asarray(inputs["b1"], f32)
    b2 = np.asarray(inputs["b2"], f32)
    ln_g = np.asarray(inputs["ln_g"], f32)
    ln_b = np.asarray(inputs["ln_b"], f32)
    Wq = np.asarray(inputs["Wq"], f32)
    bq = np.asarray(inputs["bq"], f32)
    Wk = np.asarray(inputs["Wk"], f32)
    bk = np.asarray(inputs["bk"], f32)
    Wv = np.asarray(inputs["Wv"], f32)
    bv = np.asarray(inputs["bv"], f32)
    Wp = np.asarray(inputs["Wp"], f32)

    scale = 1.0 / np.sqrt(np.float32(D))
    Wq_s = Wq * scale
    bq_s = bq * scale
    Wk_f = ln_g[:, None] * Wk
    bk_f = ln_b @ Wk + bk
    Wv_f = ln_g[:, None] * Wv
    bv_f = ln_b @ Wv + bv

    in_maps = []
    for c in range(8):
        b = c // 2
        g = c % 2
        gs = slice(g * G, (g + 1) * G)
        m = {
            "eeT": np.ascontiguousarray(ee[b].T).reshape(KC_E, P, S).astype(bf16),
            "xT": np.ascontiguousarray(x[b].T).reshape(KC_C, P, T).astype(bf16),
            "w1": W1.reshape(KC_E, P, C).astype(bf16),
            "w2": W2.reshape(KC_C, P, C).astype(bf16),
            "wq": Wq_s[:, gs].reshape(KC_C, P, G).astype(bf16),
            "wk": Wk_f[:, gs].reshape(KC_C, P, G).astype(bf16),
            "wv": Wv_f[:, gs].reshape(KC_C, P, G).astype(bf16),
            "wp": Wp[gs, :].reshape(MC_G, P, C).astype(bf16),
            "b1c": np.ascontiguousarray(b1.reshape(KC_C, P).T),
            "b2r": b2.reshape(1, C),
            "bqc": np.ascontiguousarray(bq_s[gs].reshape(MC_G, P).T),
            "bkc": np.ascontiguousarray(bk_f[gs].reshape(MC_G, P).T),
            "bvr": bv_f[gs].reshape(1, G),
            "mb": np.ascontiguousarray(
                ((mask[b].astype(f32) - 1.0) * 30.0).reshape(SC, P).T
            ),
        }
        in_maps.append(m)
    return in_maps


_NC_CACHE = None


def _get_nc():
    global _NC_CACHE
    if _NC_CACHE is None:
        _NC_CACHE = build_program()
    return _NC_CACHE


def kernel(**inputs):
    from concourse.bass_utils import run_bass_kernel_spmd

    nc = _get_nc()
    in_maps = make_in_maps(inputs)
    res = run_bass_kernel_spmd(nc, in_maps, core_ids=list(range(8)))
    bp = np.asarray(inputs["bp"], np.float32)
    outs = [res.results[c]["out"].reshape(T, C) for c in range(8)]
    y = np.stack([outs[2 * b] + outs[2 * b + 1] + bp for b in range(B)])
    return y.astype(np.float32)
